# revision 16
# baseline (speedup 1.0000x reference)
"""Grouped MLP (MoE expert FFN) Bass kernel for 8 Trainium2 NeuronCores.

Problem: 4096 tokens sorted by expert (8 experts, uneven counts), per-expert
GLU MLP:  h = x @ w1[g]  (-> up|gate, 2*2048 cols);  a = silu(up)*gate;
y = a @ w2[g].

Sharding: 2 token-groups x 4-way tensor-parallel.  Tokens split into two
equal halves; cores 0-3 process half 0, cores 4-7 half 1.  Within a group,
core tp in {0..3} owns a 512-wide slice of INTER for every expert the group
touches: fc1 column-slice (512 up + 512 gate cols), fc2 row-slice (512
rows).  Partial fc2 outputs are summed over the 4 cores of a group on the
host.  Per-core work is 512-token-equivalent for any token distribution.

Both groups must run the SAME program (SPMD), so each group's expert
segments are decomposed into a common multiset of "slots" (greedy two-list
refinement: repeatedly emit min(maxA, maxB) and carve it from both).  A
slot is one expert's weights + a fixed token count; the host binds each
group's slots to its own experts/token ranges.  For the reference counts
this gives 5 slots = 15MB of weights per core; with x (4MB) and y (4MB)
per-core DMA is ~23MB against ~82us of matmul -- DMA is no longer
co-critical with the PE.

Device program per chunk (<=512 tokens, one slot), bf16 MMs in fp32 PSUM,
feature-major:
  hT[p,half] = sum_k w1s[p,k,half]^T @ xT[k]   p in 0..3, half in {up,gt}
  hglT[p]    = silu(up_p) * gate_p             (ACT + DVE, PSUM->SBUF, bf16)
  yT[hb]     = sum_ki w2s[ki,hb]^T @ hglT[ki]  (4-term accum), cast, DMA out

All DMAs move [128 partitions x multi-KB contiguous lines].  Weights
stream in four 0.5MB fc1 pair-pieces per slot plus a 1MB fc2 piece
deferred one chunk, so the first matmul waits on only ~0.75MB.  fc2 of
chunk i is emitted after fc1 of chunk i+1 (software skew).  First/last
chunks are shrunk to 128 tokens to cut kernel ramp/tail.
"""

import sys

try:  # concourse normally comes from the container's PYTHONPATH
    import concourse  # noqa: F401
except ImportError:  # pragma: no cover - fallback for stripped env
    for _p in (
        "/root/.axon_site",
        "/root/.axon_site/_ro/trn_rl_repo",
        "/root/.axon_site/_ro/pypackages",
        "/opt/trn_rl_repo",
    ):
        if _p not in sys.path:
            sys.path.append(_p)

from contextlib import ExitStack

import numpy as np
import ml_dtypes

BF16 = np.dtype(ml_dtypes.bfloat16)

NUM_TOKENS = 4096
HIDDEN = 1024
INTER = 2048
GROUPS = 8
N_CORES = 8
N_TP = 4                       # tensor-parallel ways within a token group

SLICE = INTER // N_TP          # 512 inter cols/rows per core
N_PAIRS = SLICE // 128         # 4 (up,gt) pairs per core
KI = SLICE // 128              # 4 fc2 contraction blocks
CHUNK = 512                    # max tokens per chunk (PSUM fp32 free-dim cap)
KC = HIDDEN // 128             # 8 contraction blocks for fc1
PAIR_COLS = KC * 256           # 2048 cols per up|gt pair piece
W1_COLS = N_PAIRS * PAIR_COLS  # 8192
W2_COLS = KI * HIDDEN          # 4096
WC_COLS = W1_COLS + W2_COLS    # 12288


def _group_segments(counts):
    """Split the token stream at T/2 (expert-contiguous walk) into 2 groups.

    Returns (segs, Tg): segs[gi] = [(expert, global_tok_start, cnt), ...],
    Tg = per-group token count.  If T is odd, group 1 gets a phantom
    zero-token appended to its last segment (x zero-padded, y ignored).
    """
    T = int(sum(int(c) for c in counts))
    Tg = (T + 1) // 2
    segs = [[], []]
    pos = 0
    for g in range(GROUPS):
        cnt = int(counts[g])
        if cnt <= 0:
            continue
        s, e = pos, pos + cnt
        if e <= Tg:
            segs[0].append([g, s, cnt])
        elif s >= Tg:
            segs[1].append([g, s, cnt])
        else:
            segs[0].append([g, s, Tg - s])
            segs[1].append([g, Tg, e - Tg])
        pos = e
    if T % 2 == 1:  # phantom token to even the groups
        segs[1][-1][2] += 1
    return segs, Tg


def _common_slots(segs):
    """Greedy common refinement of the two groups' segment multisets.

    Returns slots = [(size, (expertA, tokA), (expertB, tokB)), ...] with
    sizes non-increasing; each group's piece is a contiguous token range.
    """
    A = [[cnt, g, s] for g, s, cnt in segs[0]]
    B = [[cnt, g, s] for g, s, cnt in segs[1]]
    slots = []
    while A and B:
        A.sort(key=lambda t: -t[0])
        B.sort(key=lambda t: -t[0])
        a, b = A[0], B[0]
        n = min(a[0], b[0])
        slots.append((n, (a[1], a[2]), (b[1], b[2])))
        for L, t in ((A, a), (B, b)):
            t[2] += n
            t[0] -= n
            if t[0] == 0:
                L.remove(t)
    assert not A and not B, (A, B)
    return slots


def _mk_chunks(slot_sizes):
    """Chunks (slot_idx, intra_slot_off, n<=CHUNK).

    Chunks below ~256 tokens are LDWEIGHTS-paced on the PE (~2x the
    per-token matmul cost), so never split below 256; the first chunk is
    capped at 256 only to shorten the DMA the first matmul waits on.
    """
    chunks = []
    for si, size in enumerate(slot_sizes):
        if si == 0 and size > CHUNK:
            chunks.append((si, 0, 256))
            size, o0 = size - 256, 256
        else:
            o0 = 0
        parts = -(-size // CHUNK)
        base, rem = divmod(size, parts)
        o = o0
        for i in range(parts):
            n = base + (1 if i < rem else 0)
            chunks.append((si, o, n))
            o += n
    return chunks


_PROGRAM_CACHE: dict = {}


def _build_program(key):
    """Build + compile the single-core Bass program (same NEFF on all cores).

    key = (n_slots, tuple of (slot_idx, n) per chunk).
    """
    import concourse.bass as bass  # noqa: F401
    import concourse.mybir as mybir
    import concourse.tile as tile
    from concourse import bacc

    f32 = mybir.dt.float32
    bf16 = mybir.dt.bfloat16
    silu = mybir.ActivationFunctionType.Silu

    n_slots, chunk_key = key
    Tg = sum(n for _, n in chunk_key)

    nc = bacc.Bacc("TRN2", target_bir_lowering=False, debug=False)

    x_d = nc.dram_tensor("xc", [128, KC * Tg], bf16, kind="ExternalInput").ap()
    w_d = nc.dram_tensor(
        "wc", [n_slots, 128, WC_COLS], bf16, kind="ExternalInput"
    ).ap()
    y_d = nc.dram_tensor("yc", [128, 8 * Tg], bf16, kind="ExternalOutput").ap()

    with tile.TileContext(nc) as tc, ExitStack() as ctx:
        xp = ctx.enter_context(tc.tile_pool(name="x", bufs=5))
        wp = ctx.enter_context(tc.tile_pool(name="w", bufs=3))
        hp = ctx.enter_context(tc.tile_pool(name="hgl", bufs=3))
        yp = ctx.enter_context(tc.tile_pool(name="y", bufs=3))
        tp = ctx.enter_context(tc.tile_pool(name="tmp", bufs=4))
        p1 = ctx.enter_context(tc.tile_pool(name="p1", bufs=5, space="PSUM"))
        p2 = ctx.enter_context(tc.tile_pool(name="p2", bufs=3, space="PSUM"))

        # HAM warm-up: ~5us of matmuls on scratch data while the input DMAs
        # ramp (PE is idle then anyway) so the PE clock gate is at 8/8 before
        # real work arrives instead of spending its first 3.4us at half clock
        warm_in = tp.tile([128, 640], f32, tag="warm")
        nc.vector.memset(warm_in, 0.0)
        for i in range(20):
            wps = p1.tile([128, 512], f32, tag="p1")
            nc.tensor.matmul(
                wps, warm_in[:, :128], warm_in[:, 128:640], start=True, stop=True
            )

        wt = {}          # slot -> SBUF weight tile
        w2_pending = []  # slots whose fc2 piece DMA is deferred

        def flush_w2():
            while w2_pending:
                si = w2_pending.pop(0)
                nc.sync.dma_start(
                    out=wt[si][:, W1_COLS:WC_COLS], in_=w_d[si][:, W1_COLS:WC_COLS]
                )

        def emit_fc2(si, hgl, coff, n):
            w = wt[si]
            y_sb = yp.tile([128, 8 * n], bf16, tag="y")
            for hb in range(8):
                py = p2.tile([128, n], f32, tag="p2")
                for ki in range(KI):
                    base = W1_COLS + ki * HIDDEN + hb * 128
                    nc.tensor.matmul(
                        py,
                        w[:, base : base + 128],
                        hgl[:, ki * n : (ki + 1) * n],
                        start=(ki == 0),
                        stop=(ki == KI - 1),
                    )
                dst = y_sb[:, hb * n : (hb + 1) * n]
                if hb % 2 == 0:
                    nc.scalar.copy(dst, py)
                else:
                    nc.vector.tensor_copy(dst, py)
            # outputs ride the ACT HWDGE ring so a y store blocked on its
            # casts can't head-of-line-block input DMAs on the sync ring
            nc.scalar.dma_start(out=y_d[:, 8 * coff : 8 * (coff + n)], in_=y_sb)

        pending = None  # (slot, hgl tile, core_tok_off, n) awaiting fc2
        coff = 0
        for si, n in chunk_key:
            xt = xp.tile([128, KC * n], bf16, tag="x")
            nc.sync.dma_start(out=xt, in_=x_d[:, KC * coff : KC * (coff + n)])
            if si not in wt:
                w = wp.tile([128, WC_COLS], bf16, tag="w")
                for p in range(N_PAIRS):
                    nc.sync.dma_start(
                        out=w[:, p * PAIR_COLS : (p + 1) * PAIR_COLS],
                        in_=w_d[si][:, p * PAIR_COLS : (p + 1) * PAIR_COLS],
                    )
                wt[si] = w
                deferred = True
            else:
                deferred = False

            # fc1: pair p; piece p holds k-major [up_p | gt_p]
            hgl = hp.tile([128, N_PAIRS * n], bf16, tag="h")
            for p in range(N_PAIRS):
                pu = None
                for half in (0, 1):  # 0: up, 1: gate
                    acc = p1.tile([128, n], f32, tag="p1")
                    for k in range(KC):
                        base = p * PAIR_COLS + k * 256 + half * 128
                        nc.tensor.matmul(
                            acc,
                            wt[si][:, base : base + 128],
                            xt[:, k * n : (k + 1) * n],
                            start=(k == 0),
                            stop=(k == KC - 1),
                        )
                    if half == 0:
                        pu = acc
                    else:
                        tmp = tp.tile([128, n], f32, tag="t")
                        nc.scalar.activation(tmp, pu, silu)
                        nc.vector.tensor_mul(hgl[:, p * n : (p + 1) * n], tmp, acc)

            flush_w2()
            if deferred:
                w2_pending.append(si)
            if pending is not None:
                emit_fc2(*pending)
            pending = (si, hgl, coff, n)
            coff += n

        flush_w2()
        emit_fc2(*pending)

    nc.compile()
    return nc


def _get_program(key):
    if key not in _PROGRAM_CACHE:
        _PROGRAM_CACHE[key] = _build_program(key)
    return _PROGRAM_CACHE[key]


def _chunk_map(slots, chunks, gi):
    """Per chunk: (global_tok_start, core_tok_off, n) for group gi."""
    out = []
    coff = 0
    for si, o, n in chunks:
        _, pa, pb = slots[si]
        tok0 = (pa if gi == 0 else pb)[1]
        out.append((tok0 + o, coff, n))
        coff += n
    return out


def _prep_x(xb, cmap, Tg, T):
    """bf16 [T, 1024] -> [128, 8*Tg] chunk-major k-blocked layout."""
    X = np.zeros((128, KC * Tg), BF16)
    for glob, coff, n in cmap:
        nn = min(n, T - glob)  # phantom pad tokens stay zero
        if nn <= 0:
            continue
        seg = xb[glob : glob + nn].T  # [1024, nn]
        X[:, KC * coff : KC * coff + KC * nn] = (
            seg.reshape(KC, 128, nn).transpose(1, 0, 2).reshape(128, KC * nn)
        )
    return X


def _prep_weights(w1b, w2b, slots, gi, tp_idx):
    """Per-core slot weights -> [n_slots, 128, 12288] bf16."""
    lo = tp_idx * SLICE
    wc = np.empty((len(slots), 128, WC_COLS), BF16)
    for si, (_, pa, pb) in enumerate(slots):
        e = (pa if gi == 0 else pb)[0]
        for p in range(N_PAIRS):
            u = w1b[e][:, lo + p * 128 : lo + p * 128 + 128]
            gt = w1b[e][:, INTER + lo + p * 128 : INTER + lo + p * 128 + 128]
            sl = np.concatenate([u, gt], 1)  # [1024, 256]
            wc[si, :, p * PAIR_COLS : (p + 1) * PAIR_COLS] = (
                sl.reshape(KC, 128, 256).transpose(1, 0, 2).reshape(128, PAIR_COLS)
            )
        w2s = w2b[e][lo : lo + SLICE]  # [512, 1024]
        wc[si, :, W1_COLS:] = (
            w2s.reshape(KI, 128, HIDDEN).transpose(1, 0, 2).reshape(128, W2_COLS)
        )
    return wc


_LAST_RESULTS = {}  # exposed for test.py (exec time, trace paths)


def kernel(permuted_tokens, tokens_per_expert, w1, w2, _trace=False):
    from concourse.bass_utils import run_bass_kernel_spmd

    x = np.asarray(permuted_tokens, np.float32)
    counts = np.asarray(tokens_per_expert, np.int64)
    w1 = np.asarray(w1, np.float32)
    w2 = np.asarray(w2, np.float32)
    T = int(counts.sum())

    segs, Tg = _group_segments(counts)
    slots = _common_slots(segs)
    chunks = _mk_chunks([s for s, _, _ in slots])
    key = (len(slots), tuple((si, n) for si, _, n in chunks))

    nc = _get_program(key)

    xb = x.astype(BF16)
    w1b = w1.astype(BF16)
    w2b = w2.astype(BF16)
    cmaps = [_chunk_map(slots, chunks, gi) for gi in range(2)]
    Xg = [_prep_x(xb, cmaps[gi], Tg, T) for gi in range(2)]
    in_maps = [
        {"xc": Xg[c // N_TP], "wc": _prep_weights(w1b, w2b, slots, c // N_TP, c % N_TP)}
        for c in range(N_CORES)
    ]

    kwargs = {}
    if _trace:
        kwargs = dict(trace=True, trace_cores=list(range(N_CORES)))
    res = run_bass_kernel_spmd(nc, in_maps, core_ids=list(range(N_CORES)), **kwargs)
    _LAST_RESULTS["res"] = res

    out = np.zeros((x.shape[0], HIDDEN), np.float32)
    for gi in range(2):
        acc = np.zeros((128, 8 * Tg), np.float32)
        for c in range(gi * N_TP, (gi + 1) * N_TP):
            acc += np.asarray(res.results[c]["yc"]).astype(np.float32)
        for glob, coff, n in cmaps[gi]:
            nn = min(n, T - glob)
            if nn <= 0:
                continue
            seg = acc[:, 8 * coff : 8 * coff + 8 * n].reshape(128, 8, n)[:, :, :nn]
            out[glob : glob + nn] = seg.transpose(2, 1, 0).reshape(nn, HIDDEN)
    return out


# revision 17
# speedup vs baseline: 1.1314x; 1.1314x over previous
"""Grouped MLP (MoE expert FFN) Bass kernel for 8 Trainium2 NeuronCores.

Problem: 4096 tokens sorted by expert (8 experts, uneven counts), per-expert
GLU MLP:  h = x @ w1[g]  (-> up|gate, 2*2048 cols);  a = silu(up)*gate;
y = a @ w2[g].

Sharding: 2 token-groups x 4-way tensor-parallel.  Tokens split into two
equal halves; cores 0-3 process half 0, cores 4-7 half 1.  Within a group,
core tp in {0..3} owns a 512-wide slice of INTER for every expert the group
touches: fc1 column-slice (512 up + 512 gate cols), fc2 row-slice (512
rows).  Partial fc2 outputs are summed over the 4 cores of a group on the
host.  Per-core work is 512-token-equivalent for any token distribution.

Both groups must run the SAME program (SPMD), so each group's expert
segments are decomposed into a common multiset of "slots" (greedy two-list
refinement: repeatedly emit min(maxA, maxB) and carve it from both).  A
slot is one expert's weights + a fixed token count; the host binds each
group's slots to its own experts/token ranges.  For the reference counts
this gives 5 slots = 15MB of weights per core; with x (4MB) and y (4MB)
per-core DMA is ~23MB against ~82us of matmul -- DMA is no longer
co-critical with the PE.

Device program per chunk (<=512 tokens, one slot), bf16 MMs in fp32 PSUM,
feature-major:
  hT[p,half] = sum_k w1s[p,k,half]^T @ xT[k]   p in 0..3, half in {up,gt}
  hglT[p]    = silu(up_p) * gate_p             (ACT + DVE, PSUM->SBUF, bf16)
  yT[hb]     = sum_ki w2s[ki,hb]^T @ hglT[ki]  (4-term accum), cast, DMA out

All DMAs move [128 partitions x multi-KB contiguous lines].  Weights
stream in four 0.5MB fc1 pair-pieces per slot plus a 1MB fc2 piece
deferred one chunk, so the first matmul waits on only ~0.75MB.  fc2 of
chunk i is emitted after fc1 of chunk i+1 (software skew).  First/last
chunks are shrunk to 128 tokens to cut kernel ramp/tail.
"""

import sys

try:  # concourse normally comes from the container's PYTHONPATH
    import concourse  # noqa: F401
except ImportError:  # pragma: no cover - fallback for stripped env
    for _p in (
        "/root/.axon_site",
        "/root/.axon_site/_ro/trn_rl_repo",
        "/root/.axon_site/_ro/pypackages",
        "/opt/trn_rl_repo",
    ):
        if _p not in sys.path:
            sys.path.append(_p)

from contextlib import ExitStack

import numpy as np
import ml_dtypes

BF16 = np.dtype(ml_dtypes.bfloat16)

NUM_TOKENS = 4096
HIDDEN = 1024
INTER = 2048
GROUPS = 8
N_CORES = 8
N_TP = 4                       # tensor-parallel ways within a token group

SLICE = INTER // N_TP          # 512 inter cols/rows per core
N_PAIRS = SLICE // 128         # 4 (up,gt) pairs per core
KI = SLICE // 128              # 4 fc2 contraction blocks
CHUNK = 512                    # max tokens per chunk (PSUM fp32 free-dim cap)
KC = HIDDEN // 128             # 8 contraction blocks for fc1
PAIR_COLS = KC * 256           # 2048 cols per up|gt pair piece
W1_COLS = N_PAIRS * PAIR_COLS  # 8192
W2_COLS = KI * HIDDEN          # 4096
WC_COLS = W1_COLS + W2_COLS    # 12288


def _group_segments(counts):
    """Split the token stream at T/2 (expert-contiguous walk) into 2 groups.

    Returns (segs, Tg): segs[gi] = [(expert, global_tok_start, cnt), ...],
    Tg = per-group token count.  If T is odd, group 1 gets a phantom
    zero-token appended to its last segment (x zero-padded, y ignored).
    """
    T = int(sum(int(c) for c in counts))
    Tg = (T + 1) // 2
    segs = [[], []]
    pos = 0
    for g in range(GROUPS):
        cnt = int(counts[g])
        if cnt <= 0:
            continue
        s, e = pos, pos + cnt
        if e <= Tg:
            segs[0].append([g, s, cnt])
        elif s >= Tg:
            segs[1].append([g, s, cnt])
        else:
            segs[0].append([g, s, Tg - s])
            segs[1].append([g, Tg, e - Tg])
        pos = e
    if T % 2 == 1:  # phantom token to even the groups
        segs[1][-1][2] += 1
    return segs, Tg


def _common_slots(segs):
    """Greedy common refinement of the two groups' segment multisets.

    Returns slots = [(size, (expertA, tokA), (expertB, tokB)), ...] with
    sizes non-increasing; each group's piece is a contiguous token range.
    """
    A = [[cnt, g, s] for g, s, cnt in segs[0]]
    B = [[cnt, g, s] for g, s, cnt in segs[1]]
    slots = []
    while A and B:
        A.sort(key=lambda t: -t[0])
        B.sort(key=lambda t: -t[0])
        a, b = A[0], B[0]
        n = min(a[0], b[0])
        slots.append((n, (a[1], a[2]), (b[1], b[2])))
        for L, t in ((A, a), (B, b)):
            t[2] += n
            t[0] -= n
            if t[0] == 0:
                L.remove(t)
    assert not A and not B, (A, B)
    return slots


def _mk_chunks(slot_sizes):
    """Chunks (slot_idx, intra_slot_off, n<=CHUNK).

    Chunks below ~256 tokens are LDWEIGHTS-paced on the PE (~2x the
    per-token matmul cost), so never split below 256; the first chunk is
    capped at 256 only to shorten the DMA the first matmul waits on.
    """
    chunks = []
    for si, size in enumerate(slot_sizes):
        if si == 0 and size > CHUNK:
            chunks.append((si, 0, 256))
            size, o0 = size - 256, 256
        else:
            o0 = 0
        parts = -(-size // CHUNK)
        base, rem = divmod(size, parts)
        o = o0
        for i in range(parts):
            n = base + (1 if i < rem else 0)
            chunks.append((si, o, n))
            o += n
    return chunks


_PROGRAM_CACHE: dict = {}


def _build_program(key):
    """Build + compile the single-core Bass program (same NEFF on all cores).

    key = (n_slots, tuple of (slot_idx, n) per chunk).
    """
    import concourse.bass as bass  # noqa: F401
    import concourse.mybir as mybir
    import concourse.tile as tile
    from concourse import bacc

    f32 = mybir.dt.float32
    bf16 = mybir.dt.bfloat16
    silu = mybir.ActivationFunctionType.Silu

    n_slots, chunk_key = key
    Tg = sum(n for _, n in chunk_key)

    nc = bacc.Bacc("TRN2", target_bir_lowering=False, debug=False)

    x_d = nc.dram_tensor("xc", [128, KC * Tg], bf16, kind="ExternalInput").ap()
    w_d = nc.dram_tensor(
        "wc", [n_slots, 128, WC_COLS], bf16, kind="ExternalInput"
    ).ap()
    y_d = nc.dram_tensor("yc", [128, 8 * Tg], bf16, kind="ExternalOutput").ap()

    with tile.TileContext(nc) as tc, ExitStack() as ctx:
        xp = ctx.enter_context(tc.tile_pool(name="x", bufs=5))
        wp = ctx.enter_context(tc.tile_pool(name="w", bufs=3))
        hp = ctx.enter_context(tc.tile_pool(name="hgl", bufs=3))
        yp = ctx.enter_context(tc.tile_pool(name="y", bufs=3))
        tp = ctx.enter_context(tc.tile_pool(name="tmp", bufs=4))
        p1 = ctx.enter_context(tc.tile_pool(name="p1", bufs=5, space="PSUM"))
        p2 = ctx.enter_context(tc.tile_pool(name="p2", bufs=3, space="PSUM"))

        # HAM warm-up: ~5us of matmuls on scratch data while the input DMAs
        # ramp (PE is idle then anyway) so the PE clock gate is at 8/8 before
        # real work arrives instead of spending its first 3.4us at half clock
        warm_in = tp.tile([128, 640], bf16, tag="warm")
        nc.vector.memset(warm_in, 0.0)
        for i in range(20):
            wps = p1.tile([128, 512], f32, tag="p1")
            nc.tensor.matmul(
                wps, warm_in[:, :128], warm_in[:, 128:640], start=True, stop=True
            )

        wt = {}          # slot -> SBUF weight tile
        w2_pending = []  # slots whose fc2 piece DMA is deferred

        def flush_w2():
            while w2_pending:
                si = w2_pending.pop(0)
                nc.sync.dma_start(
                    out=wt[si][:, W1_COLS:WC_COLS], in_=w_d[si][:, W1_COLS:WC_COLS]
                )

        def emit_fc2(si, hgl, coff, n):
            w = wt[si]
            y_sb = yp.tile([128, 8 * n], bf16, tag="y")
            for hb in range(8):
                py = p2.tile([128, n], f32, tag="p2")
                for ki in range(KI):
                    base = W1_COLS + ki * HIDDEN + hb * 128
                    nc.tensor.matmul(
                        py,
                        w[:, base : base + 128],
                        hgl[:, ki * n : (ki + 1) * n],
                        start=(ki == 0),
                        stop=(ki == KI - 1),
                    )
                dst = y_sb[:, hb * n : (hb + 1) * n]
                if hb % 2 == 0:
                    nc.scalar.copy(dst, py)
                else:
                    nc.vector.tensor_copy(dst, py)
            # outputs ride the ACT HWDGE ring so a y store blocked on its
            # casts can't head-of-line-block input DMAs on the sync ring
            nc.scalar.dma_start(out=y_d[:, 8 * coff : 8 * (coff + n)], in_=y_sb)

        pending = None  # (slot, hgl tile, core_tok_off, n) awaiting fc2
        coff = 0
        for si, n in chunk_key:
            xt = xp.tile([128, KC * n], bf16, tag="x")
            nc.sync.dma_start(out=xt, in_=x_d[:, KC * coff : KC * (coff + n)])
            if si not in wt:
                w = wp.tile([128, WC_COLS], bf16, tag="w")
                for p in range(N_PAIRS):
                    nc.sync.dma_start(
                        out=w[:, p * PAIR_COLS : (p + 1) * PAIR_COLS],
                        in_=w_d[si][:, p * PAIR_COLS : (p + 1) * PAIR_COLS],
                    )
                wt[si] = w
                deferred = True
            else:
                deferred = False

            # fc1: pair p; piece p holds k-major [up_p | gt_p]
            hgl = hp.tile([128, N_PAIRS * n], bf16, tag="h")
            for p in range(N_PAIRS):
                pu = None
                for half in (0, 1):  # 0: up, 1: gate
                    acc = p1.tile([128, n], f32, tag="p1")
                    for k in range(KC):
                        base = p * PAIR_COLS + k * 256 + half * 128
                        nc.tensor.matmul(
                            acc,
                            wt[si][:, base : base + 128],
                            xt[:, k * n : (k + 1) * n],
                            start=(k == 0),
                            stop=(k == KC - 1),
                        )
                    if half == 0:
                        pu = acc
                    else:
                        tmp = tp.tile([128, n], f32, tag="t")
                        nc.scalar.activation(tmp, pu, silu)
                        nc.vector.tensor_mul(hgl[:, p * n : (p + 1) * n], tmp, acc)

            flush_w2()
            if deferred:
                w2_pending.append(si)
            if pending is not None:
                emit_fc2(*pending)
            pending = (si, hgl, coff, n)
            coff += n

        flush_w2()
        emit_fc2(*pending)

    nc.compile()
    return nc


def _get_program(key):
    if key not in _PROGRAM_CACHE:
        _PROGRAM_CACHE[key] = _build_program(key)
    return _PROGRAM_CACHE[key]


def _chunk_map(slots, chunks, gi):
    """Per chunk: (global_tok_start, core_tok_off, n) for group gi."""
    out = []
    coff = 0
    for si, o, n in chunks:
        _, pa, pb = slots[si]
        tok0 = (pa if gi == 0 else pb)[1]
        out.append((tok0 + o, coff, n))
        coff += n
    return out


def _prep_x(xb, cmap, Tg, T):
    """bf16 [T, 1024] -> [128, 8*Tg] chunk-major k-blocked layout."""
    X = np.zeros((128, KC * Tg), BF16)
    for glob, coff, n in cmap:
        nn = min(n, T - glob)  # phantom pad tokens stay zero
        if nn <= 0:
            continue
        seg = xb[glob : glob + nn].T  # [1024, nn]
        X[:, KC * coff : KC * coff + KC * nn] = (
            seg.reshape(KC, 128, nn).transpose(1, 0, 2).reshape(128, KC * nn)
        )
    return X


def _prep_weights(w1b, w2b, slots, gi, tp_idx):
    """Per-core slot weights -> [n_slots, 128, 12288] bf16."""
    lo = tp_idx * SLICE
    wc = np.empty((len(slots), 128, WC_COLS), BF16)
    for si, (_, pa, pb) in enumerate(slots):
        e = (pa if gi == 0 else pb)[0]
        for p in range(N_PAIRS):
            u = w1b[e][:, lo + p * 128 : lo + p * 128 + 128]
            gt = w1b[e][:, INTER + lo + p * 128 : INTER + lo + p * 128 + 128]
            sl = np.concatenate([u, gt], 1)  # [1024, 256]
            wc[si, :, p * PAIR_COLS : (p + 1) * PAIR_COLS] = (
                sl.reshape(KC, 128, 256).transpose(1, 0, 2).reshape(128, PAIR_COLS)
            )
        w2s = w2b[e][lo : lo + SLICE]  # [512, 1024]
        wc[si, :, W1_COLS:] = (
            w2s.reshape(KI, 128, HIDDEN).transpose(1, 0, 2).reshape(128, W2_COLS)
        )
    return wc


_LAST_RESULTS = {}  # exposed for test.py (exec time, trace paths)


def kernel(permuted_tokens, tokens_per_expert, w1, w2, _trace=False):
    from concourse.bass_utils import run_bass_kernel_spmd

    x = np.asarray(permuted_tokens, np.float32)
    counts = np.asarray(tokens_per_expert, np.int64)
    w1 = np.asarray(w1, np.float32)
    w2 = np.asarray(w2, np.float32)
    T = int(counts.sum())

    segs, Tg = _group_segments(counts)
    slots = _common_slots(segs)
    chunks = _mk_chunks([s for s, _, _ in slots])
    key = (len(slots), tuple((si, n) for si, _, n in chunks))

    nc = _get_program(key)

    xb = x.astype(BF16)
    w1b = w1.astype(BF16)
    w2b = w2.astype(BF16)
    cmaps = [_chunk_map(slots, chunks, gi) for gi in range(2)]
    Xg = [_prep_x(xb, cmaps[gi], Tg, T) for gi in range(2)]
    in_maps = [
        {"xc": Xg[c // N_TP], "wc": _prep_weights(w1b, w2b, slots, c // N_TP, c % N_TP)}
        for c in range(N_CORES)
    ]

    kwargs = {}
    if _trace:
        kwargs = dict(trace=True, trace_cores=list(range(N_CORES)))
    res = run_bass_kernel_spmd(nc, in_maps, core_ids=list(range(N_CORES)), **kwargs)
    _LAST_RESULTS["res"] = res

    out = np.zeros((x.shape[0], HIDDEN), np.float32)
    for gi in range(2):
        acc = np.zeros((128, 8 * Tg), np.float32)
        for c in range(gi * N_TP, (gi + 1) * N_TP):
            acc += np.asarray(res.results[c]["yc"]).astype(np.float32)
        for glob, coff, n in cmaps[gi]:
            nn = min(n, T - glob)
            if nn <= 0:
                continue
            seg = acc[:, 8 * coff : 8 * coff + 8 * n].reshape(128, 8, n)[:, :, :nn]
            out[glob : glob + nn] = seg.transpose(2, 1, 0).reshape(nn, HIDDEN)
    return out


# revision 18
# speedup vs baseline: 1.1342x; 1.0025x over previous
"""Grouped MLP (MoE expert FFN) Bass kernel for 8 Trainium2 NeuronCores.

Problem: 4096 tokens sorted by expert (8 experts, uneven counts), per-expert
GLU MLP:  h = x @ w1[g]  (-> up|gate, 2*2048 cols);  a = silu(up)*gate;
y = a @ w2[g].

Sharding: 2 token-groups x 4-way tensor-parallel.  Tokens split into two
equal halves; cores 0-3 process half 0, cores 4-7 half 1.  Within a group,
core tp in {0..3} owns a 512-wide slice of INTER for every expert the group
touches: fc1 column-slice (512 up + 512 gate cols), fc2 row-slice (512
rows).  Partial fc2 outputs are summed over the 4 cores of a group on the
host.  Per-core work is 512-token-equivalent for any token distribution.

Both groups must run the SAME program (SPMD), so each group's expert
segments are decomposed into a common multiset of "slots" (greedy two-list
refinement: repeatedly emit min(maxA, maxB) and carve it from both).  A
slot is one expert's weights + a fixed token count; the host binds each
group's slots to its own experts/token ranges.  For the reference counts
this gives 5 slots = 15MB of weights per core; with x (4MB) and y (4MB)
per-core DMA is ~23MB against ~82us of matmul -- DMA is no longer
co-critical with the PE.

Device program per chunk (<=512 tokens, one slot), bf16 MMs in fp32 PSUM,
feature-major:
  hT[p,half] = sum_k w1s[p,k,half]^T @ xT[k]   p in 0..3, half in {up,gt}
  hglT[p]    = silu(up_p) * gate_p             (ACT + DVE, PSUM->SBUF, bf16)
  yT[hb]     = sum_ki w2s[ki,hb]^T @ hglT[ki]  (4-term accum), cast, DMA out

All DMAs move [128 partitions x multi-KB contiguous lines].  Weights
stream in four 0.5MB fc1 pair-pieces per slot plus a 1MB fc2 piece
deferred one chunk, so the first matmul waits on only ~0.75MB.  fc2 of
chunk i is emitted after fc1 of chunk i+1 (software skew).  First/last
chunks are shrunk to 128 tokens to cut kernel ramp/tail.
"""

import sys

try:  # concourse normally comes from the container's PYTHONPATH
    import concourse  # noqa: F401
except ImportError:  # pragma: no cover - fallback for stripped env
    for _p in (
        "/root/.axon_site",
        "/root/.axon_site/_ro/trn_rl_repo",
        "/root/.axon_site/_ro/pypackages",
        "/opt/trn_rl_repo",
    ):
        if _p not in sys.path:
            sys.path.append(_p)

from contextlib import ExitStack

import numpy as np
import ml_dtypes

BF16 = np.dtype(ml_dtypes.bfloat16)

NUM_TOKENS = 4096
HIDDEN = 1024
INTER = 2048
GROUPS = 8
N_CORES = 8
N_TP = 4                       # tensor-parallel ways within a token group

SLICE = INTER // N_TP          # 512 inter cols/rows per core
N_PAIRS = SLICE // 128         # 4 (up,gt) pairs per core
KI = SLICE // 128              # 4 fc2 contraction blocks
CHUNK = 512                    # max tokens per chunk (PSUM fp32 free-dim cap)
KC = HIDDEN // 128             # 8 contraction blocks for fc1
PAIR_COLS = KC * 256           # 2048 cols per up|gt pair piece
W1_COLS = N_PAIRS * PAIR_COLS  # 8192
W2_COLS = KI * HIDDEN          # 4096
WC_COLS = W1_COLS + W2_COLS    # 12288


def _group_segments(counts):
    """Split the token stream at T/2 (expert-contiguous walk) into 2 groups.

    Returns (segs, Tg): segs[gi] = [(expert, global_tok_start, cnt), ...],
    Tg = per-group token count.  If T is odd, group 1 gets a phantom
    zero-token appended to its last segment (x zero-padded, y ignored).
    """
    T = int(sum(int(c) for c in counts))
    Tg = (T + 1) // 2
    segs = [[], []]
    pos = 0
    for g in range(GROUPS):
        cnt = int(counts[g])
        if cnt <= 0:
            continue
        s, e = pos, pos + cnt
        if e <= Tg:
            segs[0].append([g, s, cnt])
        elif s >= Tg:
            segs[1].append([g, s, cnt])
        else:
            segs[0].append([g, s, Tg - s])
            segs[1].append([g, Tg, e - Tg])
        pos = e
    if T % 2 == 1:  # phantom token to even the groups
        segs[1][-1][2] += 1
    return segs, Tg


def _common_slots(segs):
    """Greedy common refinement of the two groups' segment multisets.

    Returns slots = [(size, (expertA, tokA), (expertB, tokB)), ...] with
    sizes non-increasing; each group's piece is a contiguous token range.
    """
    A = [[cnt, g, s] for g, s, cnt in segs[0]]
    B = [[cnt, g, s] for g, s, cnt in segs[1]]
    slots = []
    while A and B:
        A.sort(key=lambda t: -t[0])
        B.sort(key=lambda t: -t[0])
        a, b = A[0], B[0]
        n = min(a[0], b[0])
        # chunks under 256 tokens are LDWEIGHTS-paced on the PE (~2x cost
        # per token), so shrink the slot rather than strand a tiny remainder
        for _ in range(4):
            ra, rb = a[0] - n, b[0] - n
            if 0 < ra < 256:
                n = a[0] - 256
            elif 0 < rb < 256:
                n = b[0] - 256
            else:
                break
        if n <= 0:
            n = min(a[0], b[0])
        slots.append((n, (a[1], a[2]), (b[1], b[2])))
        for L, t in ((A, a), (B, b)):
            t[2] += n
            t[0] -= n
            if t[0] == 0:
                L.remove(t)
    assert not A and not B, (A, B)
    return slots


def _mk_chunks(slot_sizes):
    """Chunks (slot_idx, intra_slot_off, n<=CHUNK).

    Chunks below ~256 tokens are LDWEIGHTS-paced on the PE (~2x the
    per-token matmul cost), so never split below 256; the first chunk is
    capped at 256 only to shorten the DMA the first matmul waits on.
    """
    chunks = []
    for si, size in enumerate(slot_sizes):
        if si == 0 and size > CHUNK:
            chunks.append((si, 0, 256))
            size, o0 = size - 256, 256
        else:
            o0 = 0
        parts = -(-size // CHUNK)
        base, rem = divmod(size, parts)
        o = o0
        for i in range(parts):
            n = base + (1 if i < rem else 0)
            chunks.append((si, o, n))
            o += n
    return chunks


_PROGRAM_CACHE: dict = {}


def _build_program(key):
    """Build + compile the single-core Bass program (same NEFF on all cores).

    key = (n_slots, tuple of (slot_idx, n) per chunk).
    """
    import concourse.bass as bass  # noqa: F401
    import concourse.mybir as mybir
    import concourse.tile as tile
    from concourse import bacc

    f32 = mybir.dt.float32
    bf16 = mybir.dt.bfloat16
    silu = mybir.ActivationFunctionType.Silu

    n_slots, chunk_key = key
    Tg = sum(n for _, n in chunk_key)

    nc = bacc.Bacc("TRN2", target_bir_lowering=False, debug=False)

    x_d = nc.dram_tensor("xc", [128, KC * Tg], bf16, kind="ExternalInput").ap()
    w_d = nc.dram_tensor(
        "wc", [n_slots, 128, WC_COLS], bf16, kind="ExternalInput"
    ).ap()
    y_d = nc.dram_tensor("yc", [128, 8 * Tg], bf16, kind="ExternalOutput").ap()

    with tile.TileContext(nc) as tc, ExitStack() as ctx:
        xp = ctx.enter_context(tc.tile_pool(name="x", bufs=5))
        wp = ctx.enter_context(tc.tile_pool(name="w", bufs=3))
        hp = ctx.enter_context(tc.tile_pool(name="hgl", bufs=3))
        yp = ctx.enter_context(tc.tile_pool(name="y", bufs=3))
        tp = ctx.enter_context(tc.tile_pool(name="tmp", bufs=4))
        p1 = ctx.enter_context(tc.tile_pool(name="p1", bufs=5, space="PSUM"))
        p2 = ctx.enter_context(tc.tile_pool(name="p2", bufs=3, space="PSUM"))

        # HAM warm-up: ~5us of matmuls on scratch data while the input DMAs
        # ramp (PE is idle then anyway) so the PE clock gate is at 8/8 before
        # real work arrives instead of spending its first 3.4us at half clock
        warm_in = tp.tile([128, 640], bf16, tag="warm")
        nc.vector.memset(warm_in, 0.0)
        for i in range(20):
            wps = p1.tile([128, 512], f32, tag="p1")
            nc.tensor.matmul(
                wps, warm_in[:, :128], warm_in[:, 128:640], start=True, stop=True
            )

        wt = {}          # slot -> SBUF weight tile
        w2_pending = []  # slots whose fc2 piece DMA is deferred

        def flush_w2():
            while w2_pending:
                si = w2_pending.pop(0)
                nc.sync.dma_start(
                    out=wt[si][:, W1_COLS:WC_COLS], in_=w_d[si][:, W1_COLS:WC_COLS]
                )

        def emit_fc2(si, hgl, coff, n):
            w = wt[si]
            y_sb = yp.tile([128, 8 * n], bf16, tag="y")
            for hb in range(8):
                py = p2.tile([128, n], f32, tag="p2")
                for ki in range(KI):
                    base = W1_COLS + ki * HIDDEN + hb * 128
                    nc.tensor.matmul(
                        py,
                        w[:, base : base + 128],
                        hgl[:, ki * n : (ki + 1) * n],
                        start=(ki == 0),
                        stop=(ki == KI - 1),
                    )
                dst = y_sb[:, hb * n : (hb + 1) * n]
                if hb % 2 == 0:
                    nc.scalar.copy(dst, py)
                else:
                    nc.vector.tensor_copy(dst, py)
            # outputs ride the ACT HWDGE ring so a y store blocked on its
            # casts can't head-of-line-block input DMAs on the sync ring
            nc.scalar.dma_start(out=y_d[:, 8 * coff : 8 * (coff + n)], in_=y_sb)

        pending = None  # (slot, hgl tile, core_tok_off, n) awaiting fc2
        coff = 0
        for si, n in chunk_key:
            xt = xp.tile([128, KC * n], bf16, tag="x")
            nc.sync.dma_start(out=xt, in_=x_d[:, KC * coff : KC * (coff + n)])
            if si not in wt:
                w = wp.tile([128, WC_COLS], bf16, tag="w")
                for p in range(N_PAIRS):
                    nc.sync.dma_start(
                        out=w[:, p * PAIR_COLS : (p + 1) * PAIR_COLS],
                        in_=w_d[si][:, p * PAIR_COLS : (p + 1) * PAIR_COLS],
                    )
                wt[si] = w
                deferred = True
            else:
                deferred = False

            # fc1: pair p; piece p holds k-major [up_p | gt_p]
            hgl = hp.tile([128, N_PAIRS * n], bf16, tag="h")
            for p in range(N_PAIRS):
                pu = None
                for half in (0, 1):  # 0: up, 1: gate
                    acc = p1.tile([128, n], f32, tag="p1")
                    for k in range(KC):
                        base = p * PAIR_COLS + k * 256 + half * 128
                        nc.tensor.matmul(
                            acc,
                            wt[si][:, base : base + 128],
                            xt[:, k * n : (k + 1) * n],
                            start=(k == 0),
                            stop=(k == KC - 1),
                        )
                    if half == 0:
                        pu = acc
                    else:
                        tmp = tp.tile([128, n], f32, tag="t")
                        nc.scalar.activation(tmp, pu, silu)
                        nc.vector.tensor_mul(hgl[:, p * n : (p + 1) * n], tmp, acc)

            flush_w2()
            if deferred:
                w2_pending.append(si)
            if pending is not None:
                emit_fc2(*pending)
            pending = (si, hgl, coff, n)
            coff += n

        flush_w2()
        emit_fc2(*pending)

    nc.compile()
    return nc


def _get_program(key):
    if key not in _PROGRAM_CACHE:
        _PROGRAM_CACHE[key] = _build_program(key)
    return _PROGRAM_CACHE[key]


def _chunk_map(slots, chunks, gi):
    """Per chunk: (global_tok_start, core_tok_off, n) for group gi."""
    out = []
    coff = 0
    for si, o, n in chunks:
        _, pa, pb = slots[si]
        tok0 = (pa if gi == 0 else pb)[1]
        out.append((tok0 + o, coff, n))
        coff += n
    return out


def _prep_x(xb, cmap, Tg, T):
    """bf16 [T, 1024] -> [128, 8*Tg] chunk-major k-blocked layout."""
    X = np.zeros((128, KC * Tg), BF16)
    for glob, coff, n in cmap:
        nn = min(n, T - glob)  # phantom pad tokens stay zero
        if nn <= 0:
            continue
        seg = xb[glob : glob + nn].T  # [1024, nn]
        X[:, KC * coff : KC * coff + KC * nn] = (
            seg.reshape(KC, 128, nn).transpose(1, 0, 2).reshape(128, KC * nn)
        )
    return X


def _prep_weights(w1b, w2b, slots, gi, tp_idx):
    """Per-core slot weights -> [n_slots, 128, 12288] bf16."""
    lo = tp_idx * SLICE
    wc = np.empty((len(slots), 128, WC_COLS), BF16)
    for si, (_, pa, pb) in enumerate(slots):
        e = (pa if gi == 0 else pb)[0]
        for p in range(N_PAIRS):
            u = w1b[e][:, lo + p * 128 : lo + p * 128 + 128]
            gt = w1b[e][:, INTER + lo + p * 128 : INTER + lo + p * 128 + 128]
            sl = np.concatenate([u, gt], 1)  # [1024, 256]
            wc[si, :, p * PAIR_COLS : (p + 1) * PAIR_COLS] = (
                sl.reshape(KC, 128, 256).transpose(1, 0, 2).reshape(128, PAIR_COLS)
            )
        w2s = w2b[e][lo : lo + SLICE]  # [512, 1024]
        wc[si, :, W1_COLS:] = (
            w2s.reshape(KI, 128, HIDDEN).transpose(1, 0, 2).reshape(128, W2_COLS)
        )
    return wc


_LAST_RESULTS = {}  # exposed for test.py (exec time, trace paths)


def kernel(permuted_tokens, tokens_per_expert, w1, w2, _trace=False):
    from concourse.bass_utils import run_bass_kernel_spmd

    x = np.asarray(permuted_tokens, np.float32)
    counts = np.asarray(tokens_per_expert, np.int64)
    w1 = np.asarray(w1, np.float32)
    w2 = np.asarray(w2, np.float32)
    T = int(counts.sum())

    segs, Tg = _group_segments(counts)
    slots = _common_slots(segs)
    chunks = _mk_chunks([s for s, _, _ in slots])
    key = (len(slots), tuple((si, n) for si, _, n in chunks))

    nc = _get_program(key)

    xb = x.astype(BF16)
    w1b = w1.astype(BF16)
    w2b = w2.astype(BF16)
    cmaps = [_chunk_map(slots, chunks, gi) for gi in range(2)]
    Xg = [_prep_x(xb, cmaps[gi], Tg, T) for gi in range(2)]
    in_maps = [
        {"xc": Xg[c // N_TP], "wc": _prep_weights(w1b, w2b, slots, c // N_TP, c % N_TP)}
        for c in range(N_CORES)
    ]

    kwargs = {}
    if _trace:
        kwargs = dict(trace=True, trace_cores=list(range(N_CORES)))
    res = run_bass_kernel_spmd(nc, in_maps, core_ids=list(range(N_CORES)), **kwargs)
    _LAST_RESULTS["res"] = res

    out = np.zeros((x.shape[0], HIDDEN), np.float32)
    for gi in range(2):
        acc = np.zeros((128, 8 * Tg), np.float32)
        for c in range(gi * N_TP, (gi + 1) * N_TP):
            acc += np.asarray(res.results[c]["yc"]).astype(np.float32)
        for glob, coff, n in cmaps[gi]:
            nn = min(n, T - glob)
            if nn <= 0:
                continue
            seg = acc[:, 8 * coff : 8 * coff + 8 * n].reshape(128, 8, n)[:, :, :nn]
            out[glob : glob + nn] = seg.transpose(2, 1, 0).reshape(nn, HIDDEN)
    return out


# revision 21
# speedup vs baseline: 1.1525x; 1.0161x over previous
"""Grouped MLP (MoE expert FFN) Bass kernel for 8 Trainium2 NeuronCores.

Problem: 4096 tokens sorted by expert (8 experts, uneven counts), per-expert
GLU MLP:  h = x @ w1[g]  (-> up|gate, 2*2048 cols);  a = silu(up)*gate;
y = a @ w2[g].

Sharding: 2 token-groups x 4-way tensor-parallel.  Tokens split into two
equal halves; cores 0-3 process half 0, cores 4-7 half 1.  Within a group,
core tp in {0..3} owns a 512-wide slice of INTER for every expert the group
touches: fc1 column-slice (512 up + 512 gate cols), fc2 row-slice (512
rows).  Partial fc2 outputs are summed over the 4 cores of a group on the
host.  Per-core work is 512-token-equivalent for any token distribution.

Both groups must run the SAME program (SPMD), so each group's expert
segments are decomposed into a common multiset of "slots" (greedy two-list
refinement: repeatedly emit min(maxA, maxB) and carve it from both).  A
slot is one expert's weights + a fixed token count; the host binds each
group's slots to its own experts/token ranges.  For the reference counts
this gives 5 slots = 15MB of weights per core; with x (4MB) and y (4MB)
per-core DMA is ~23MB against ~82us of matmul -- DMA is no longer
co-critical with the PE.

Device program per chunk (<=512 tokens, one slot), bf16 MMs in fp32 PSUM,
feature-major:
  hT[p,half] = sum_k w1s[p,k,half]^T @ xT[k]   p in 0..3, half in {up,gt}
  hglT[p]    = silu(up_p) * gate_p             (ACT + DVE, PSUM->SBUF, bf16)
  yT[hb]     = sum_ki w2s[ki,hb]^T @ hglT[ki]  (4-term accum), cast, DMA out

All DMAs move [128 partitions x multi-KB contiguous lines].  Weights
stream in four 0.5MB fc1 pair-pieces per slot plus a 1MB fc2 piece
deferred one chunk, so the first matmul waits on only ~0.75MB.  fc2 of
chunk i is emitted after fc1 of chunk i+1 (software skew).  First/last
chunks are shrunk to 128 tokens to cut kernel ramp/tail.
"""

import sys

try:  # concourse normally comes from the container's PYTHONPATH
    import concourse  # noqa: F401
except ImportError:  # pragma: no cover - fallback for stripped env
    for _p in (
        "/root/.axon_site",
        "/root/.axon_site/_ro/trn_rl_repo",
        "/root/.axon_site/_ro/pypackages",
        "/opt/trn_rl_repo",
    ):
        if _p not in sys.path:
            sys.path.append(_p)

from contextlib import ExitStack

import numpy as np
import ml_dtypes

BF16 = np.dtype(ml_dtypes.bfloat16)

NUM_TOKENS = 4096
HIDDEN = 1024
INTER = 2048
GROUPS = 8
N_CORES = 8
N_TP = 4                       # tensor-parallel ways within a token group

SLICE = INTER // N_TP          # 512 inter cols/rows per core
N_PAIRS = SLICE // 128         # 4 (up,gt) pairs per core
KI = SLICE // 128              # 4 fc2 contraction blocks
CHUNK = 512                    # max tokens per chunk (PSUM fp32 free-dim cap)
KC = HIDDEN // 128             # 8 contraction blocks for fc1
PAIR_COLS = KC * 256           # 2048 cols per up|gt pair piece
W1_COLS = N_PAIRS * PAIR_COLS  # 8192
W2_COLS = KI * HIDDEN          # 4096
WC_COLS = W1_COLS + W2_COLS    # 12288


def _group_segments(counts):
    """Split the token stream at T/2 (expert-contiguous walk) into 2 groups.

    Returns (segs, Tg): segs[gi] = [(expert, global_tok_start, cnt), ...],
    Tg = per-group token count.  If T is odd, group 1 gets a phantom
    zero-token appended to its last segment (x zero-padded, y ignored).
    """
    T = int(sum(int(c) for c in counts))
    Tg = (T + 1) // 2
    segs = [[], []]
    pos = 0
    for g in range(GROUPS):
        cnt = int(counts[g])
        if cnt <= 0:
            continue
        s, e = pos, pos + cnt
        if e <= Tg:
            segs[0].append([g, s, cnt])
        elif s >= Tg:
            segs[1].append([g, s, cnt])
        else:
            segs[0].append([g, s, Tg - s])
            segs[1].append([g, Tg, e - Tg])
        pos = e
    if T % 2 == 1:  # phantom token to even the groups
        segs[1][-1][2] += 1
    return segs, Tg


def _common_slots(segs):
    """Greedy common refinement of the two groups' segment multisets.

    Returns slots = [(size, (expertA, tokA), (expertB, tokB)), ...] with
    sizes non-increasing; each group's piece is a contiguous token range.
    """
    A = [[cnt, g, s] for g, s, cnt in segs[0]]
    B = [[cnt, g, s] for g, s, cnt in segs[1]]
    slots = []
    while A and B:
        A.sort(key=lambda t: -t[0])
        B.sort(key=lambda t: -t[0])
        a, b = A[0], B[0]
        n = min(a[0], b[0])
        # chunks under 256 tokens are LDWEIGHTS-paced on the PE (~2x cost
        # per token), so shrink the slot rather than strand a tiny remainder
        for _ in range(4):
            ra, rb = a[0] - n, b[0] - n
            if 0 < ra < 256:
                n = a[0] - 256
            elif 0 < rb < 256:
                n = b[0] - 256
            else:
                break
        if n <= 0:
            n = min(a[0], b[0])
        slots.append((n, (a[1], a[2]), (b[1], b[2])))
        for L, t in ((A, a), (B, b)):
            t[2] += n
            t[0] -= n
            if t[0] == 0:
                L.remove(t)
    assert not A and not B, (A, B)
    return slots


def _mk_chunks(slot_sizes):
    """Chunks (slot_idx, intra_slot_off, n<=CHUNK).

    Chunks below ~256 tokens are LDWEIGHTS-paced on the PE (~2x the
    per-token matmul cost), so never split below 256; the first chunk is
    capped at 256 only to shorten the DMA the first matmul waits on.
    """
    chunks = []
    for si, size in enumerate(slot_sizes):
        if si == 0 and size > CHUNK:
            chunks.append((si, 0, 256))
            size, o0 = size - 256, 256
        else:
            o0 = 0
        parts = -(-size // CHUNK)
        base, rem = divmod(size, parts)
        o = o0
        for i in range(parts):
            n = base + (1 if i < rem else 0)
            chunks.append((si, o, n))
            o += n
    return chunks


_PROGRAM_CACHE: dict = {}


def _build_program(key):
    """Build + compile the single-core Bass program (same NEFF on all cores).

    key = (n_slots, tuple of (slot_idx, n) per chunk).
    """
    import concourse.bass as bass  # noqa: F401
    import concourse.mybir as mybir
    import concourse.tile as tile
    from concourse import bacc

    f32 = mybir.dt.float32
    bf16 = mybir.dt.bfloat16
    silu = mybir.ActivationFunctionType.Silu

    n_slots, chunk_key = key
    Tg = sum(n for _, n in chunk_key)

    nc = bacc.Bacc("TRN2", target_bir_lowering=False, debug=False)

    x_d = nc.dram_tensor("xc", [128, KC * Tg], bf16, kind="ExternalInput").ap()
    w_d = nc.dram_tensor(
        "wc", [n_slots, 128, WC_COLS], bf16, kind="ExternalInput"
    ).ap()
    y_d = nc.dram_tensor("yc", [128, 8 * Tg], bf16, kind="ExternalOutput").ap()

    with tile.TileContext(nc) as tc, ExitStack() as ctx:
        xp = ctx.enter_context(tc.tile_pool(name="x", bufs=5))
        wp = ctx.enter_context(tc.tile_pool(name="w", bufs=3))
        hp = ctx.enter_context(tc.tile_pool(name="hgl", bufs=3))
        yp = ctx.enter_context(tc.tile_pool(name="y", bufs=3))
        tp = ctx.enter_context(tc.tile_pool(name="tmp", bufs=4))
        p1 = ctx.enter_context(tc.tile_pool(name="p1", bufs=5, space="PSUM"))
        p2 = ctx.enter_context(tc.tile_pool(name="p2", bufs=3, space="PSUM"))

        # HAM warm-up: ~5us of matmuls on scratch data while the input DMAs
        # ramp (PE is idle then anyway) so the PE clock gate is at 8/8 before
        # real work arrives instead of spending its first 3.4us at half clock
        warm_in = tp.tile([128, 640], bf16, tag="warm")
        nc.vector.memset(warm_in, 0.0)
        for i in range(20):
            wps = p1.tile([128, 512], f32, tag="p1")
            nc.tensor.matmul(
                wps, warm_in[:, :128], warm_in[:, 128:640], start=True, stop=True
            )

        wt = {}          # slot -> SBUF weight tile
        w2_pending = []  # slots whose fc2 piece DMA is deferred

        def flush_w2():
            while w2_pending:
                si = w2_pending.pop(0)
                nc.sync.dma_start(
                    out=wt[si][:, W1_COLS:WC_COLS], in_=w_d[si][:, W1_COLS:WC_COLS]
                )

        def emit_fc2(si, hgl, coff, n):
            w = wt[si]
            y_sb = yp.tile([128, 8 * n], bf16, tag="y")
            for hb in range(8):
                py = p2.tile([128, n], f32, tag="p2")
                for ki in range(KI):
                    base = W1_COLS + ki * HIDDEN + hb * 128
                    nc.tensor.matmul(
                        py,
                        w[:, base : base + 128],
                        hgl[:, ki * n : (ki + 1) * n],
                        start=(ki == 0),
                        stop=(ki == KI - 1),
                    )
                dst = y_sb[:, hb * n : (hb + 1) * n]
                if hb % 2 == 0:
                    nc.scalar.copy(dst, py)
                else:
                    nc.vector.tensor_copy(dst, py)
                if hb == 3:
                    # first half out early: overlaps the second half's fc2
                    # (matters for the very last chunk, where nothing else
                    # hides the store)
                    nc.scalar.dma_start(
                        out=y_d[:, 8 * coff : 8 * coff + 4 * n], in_=y_sb[:, : 4 * n]
                    )
            # outputs ride the ACT HWDGE ring so a y store blocked on its
            # casts can't head-of-line-block input DMAs on the sync ring
            nc.scalar.dma_start(
                out=y_d[:, 8 * coff + 4 * n : 8 * (coff + n)], in_=y_sb[:, 4 * n :]
            )

        pending = None  # (slot, hgl tile, core_tok_off, n) awaiting fc2
        coff = 0
        first = True
        for si, n in chunk_key:
            xt = xp.tile([128, KC * n], bf16, tag="x")
            # first chunk: halve the transfers the very first matmuls wait on
            # (the DMA path runs well below peak while it warms up)
            x_pieces = 2 if first else 1
            for h in range(x_pieces):
                lo = h * (KC // x_pieces)
                hi = (h + 1) * (KC // x_pieces)
                nc.sync.dma_start(
                    out=xt[:, lo * n : hi * n],
                    in_=x_d[:, KC * coff + lo * n : KC * coff + hi * n],
                )
            if si not in wt:
                w = wp.tile([128, WC_COLS], bf16, tag="w")
                for p in range(N_PAIRS):
                    w_pieces = 2 if first and p == 0 else 1
                    for h in range(w_pieces):
                        c0 = p * PAIR_COLS + h * (PAIR_COLS // w_pieces)
                        c1 = c0 + PAIR_COLS // w_pieces
                        nc.sync.dma_start(
                            out=w[:, c0:c1], in_=w_d[si][:, c0:c1]
                        )
                wt[si] = w
                deferred = True
            else:
                deferred = False
            first = False

            # fc1: pair p; piece p holds k-major [up_p | gt_p]
            hgl = hp.tile([128, N_PAIRS * n], bf16, tag="h")
            for p in range(N_PAIRS):
                pu = None
                for half in (0, 1):  # 0: up, 1: gate
                    acc = p1.tile([128, n], f32, tag="p1")
                    for k in range(KC):
                        base = p * PAIR_COLS + k * 256 + half * 128
                        nc.tensor.matmul(
                            acc,
                            wt[si][:, base : base + 128],
                            xt[:, k * n : (k + 1) * n],
                            start=(k == 0),
                            stop=(k == KC - 1),
                        )
                    if half == 0:
                        pu = acc
                    else:
                        tmp = tp.tile([128, n], f32, tag="t")
                        nc.scalar.activation(tmp, pu, silu)
                        nc.vector.tensor_mul(hgl[:, p * n : (p + 1) * n], tmp, acc)

            flush_w2()
            if deferred:
                w2_pending.append(si)
            if pending is not None:
                emit_fc2(*pending)
            pending = (si, hgl, coff, n)
            coff += n

        flush_w2()
        emit_fc2(*pending)

    nc.compile()
    return nc


def _get_program(key):
    if key not in _PROGRAM_CACHE:
        _PROGRAM_CACHE[key] = _build_program(key)
    return _PROGRAM_CACHE[key]


def _chunk_map(slots, chunks, gi):
    """Per chunk: (global_tok_start, core_tok_off, n) for group gi."""
    out = []
    coff = 0
    for si, o, n in chunks:
        _, pa, pb = slots[si]
        tok0 = (pa if gi == 0 else pb)[1]
        out.append((tok0 + o, coff, n))
        coff += n
    return out


def _prep_x(xb, cmap, Tg, T):
    """bf16 [T, 1024] -> [128, 8*Tg] chunk-major k-blocked layout."""
    X = np.zeros((128, KC * Tg), BF16)
    for glob, coff, n in cmap:
        nn = min(n, T - glob)  # phantom pad tokens stay zero
        if nn <= 0:
            continue
        seg = xb[glob : glob + nn].T  # [1024, nn]
        X[:, KC * coff : KC * coff + KC * nn] = (
            seg.reshape(KC, 128, nn).transpose(1, 0, 2).reshape(128, KC * nn)
        )
    return X


def _prep_weights(w1b, w2b, slots, gi, tp_idx):
    """Per-core slot weights -> [n_slots, 128, 12288] bf16."""
    lo = tp_idx * SLICE
    wc = np.empty((len(slots), 128, WC_COLS), BF16)
    for si, (_, pa, pb) in enumerate(slots):
        e = (pa if gi == 0 else pb)[0]
        for p in range(N_PAIRS):
            u = w1b[e][:, lo + p * 128 : lo + p * 128 + 128]
            gt = w1b[e][:, INTER + lo + p * 128 : INTER + lo + p * 128 + 128]
            sl = np.concatenate([u, gt], 1)  # [1024, 256]
            wc[si, :, p * PAIR_COLS : (p + 1) * PAIR_COLS] = (
                sl.reshape(KC, 128, 256).transpose(1, 0, 2).reshape(128, PAIR_COLS)
            )
        w2s = w2b[e][lo : lo + SLICE]  # [512, 1024]
        wc[si, :, W1_COLS:] = (
            w2s.reshape(KI, 128, HIDDEN).transpose(1, 0, 2).reshape(128, W2_COLS)
        )
    return wc


_LAST_RESULTS = {}  # exposed for test.py (exec time, trace paths)


def kernel(permuted_tokens, tokens_per_expert, w1, w2, _trace=False):
    from concourse.bass_utils import run_bass_kernel_spmd

    x = np.asarray(permuted_tokens, np.float32)
    counts = np.asarray(tokens_per_expert, np.int64)
    w1 = np.asarray(w1, np.float32)
    w2 = np.asarray(w2, np.float32)
    T = int(counts.sum())

    segs, Tg = _group_segments(counts)
    slots = _common_slots(segs)
    chunks = _mk_chunks([s for s, _, _ in slots])
    key = (len(slots), tuple((si, n) for si, _, n in chunks))

    nc = _get_program(key)

    xb = x.astype(BF16)
    w1b = w1.astype(BF16)
    w2b = w2.astype(BF16)
    cmaps = [_chunk_map(slots, chunks, gi) for gi in range(2)]
    Xg = [_prep_x(xb, cmaps[gi], Tg, T) for gi in range(2)]
    in_maps = [
        {"xc": Xg[c // N_TP], "wc": _prep_weights(w1b, w2b, slots, c // N_TP, c % N_TP)}
        for c in range(N_CORES)
    ]

    kwargs = {}
    if _trace:
        kwargs = dict(trace=True, trace_cores=list(range(N_CORES)))
    res = run_bass_kernel_spmd(nc, in_maps, core_ids=list(range(N_CORES)), **kwargs)
    _LAST_RESULTS["res"] = res

    out = np.zeros((x.shape[0], HIDDEN), np.float32)
    for gi in range(2):
        acc = np.zeros((128, 8 * Tg), np.float32)
        for c in range(gi * N_TP, (gi + 1) * N_TP):
            acc += np.asarray(res.results[c]["yc"]).astype(np.float32)
        for glob, coff, n in cmaps[gi]:
            nn = min(n, T - glob)
            if nn <= 0:
                continue
            seg = acc[:, 8 * coff : 8 * coff + 8 * n].reshape(128, 8, n)[:, :, :nn]
            out[glob : glob + nn] = seg.transpose(2, 1, 0).reshape(nn, HIDDEN)
    return out
